# revision 9
# baseline (speedup 1.0000x reference)
"""Trainium2 Bass kernel for the traffic-GNN (GRU + 2 GAT layers).

Sharding: nodes split contiguously across 8 cores (6250 each, padded to 6272
= 49 blocks of 128). Edges partitioned by destination core/block (host-side
sort = sharding prep). Per-core node features are computed locally
(feature-major), packed into gather tables, AllGathered, then each core runs
its edge blocks: dma_gather of source rows, edge softmax (no max-subtraction
-- logits are O(1)), and a one-hot-matmul segment-sum into PSUM.
"""
import sys
import numpy as np

# ---------------- problem constants (full size) ----------------
FULL_CFG = dict(
    N=50000, E=800000, CORES=8,
    NODES_PC=6250, NB=49,           # blocks of 128 dsts per core
    CT=448,                          # node column tile (<=512)
    SPLIT=32768,                     # int16 gather table split
)
STATIC, T, RNN_H, GNN_H, TE = 16, 12, 64, 64, 16
P = 128
D1 = 320          # layer-1 table row f32 (264 used, padded to 1280B)
D2 = 128          # layer-2 table row f32 (66 used, padded to 512B)


def _wrap16(a):
    """dma_gather index layout: idx i -> [i%16, i//16], replicated to 128."""
    w = a.reshape(-1, 16).T.copy()
    return np.tile(w, (8, 1))


def host_prep(inputs, cfg):
    """Build per-core input arrays + compile-time metadata."""
    N, E, CORES = cfg["N"], cfg["E"], cfg["CORES"]
    NPC, NB, CT, SPLIT = cfg["NODES_PC"], cfg["NB"], cfg["CT"], cfg["SPLIT"]
    NPAD = NB * P
    HALF = NPAD // 2
    PAIRS = HALF // CT

    f32 = np.float32
    xs = np.asarray(inputs["x_static"], f32)
    hs = np.asarray(inputs["hist_speed"], f32).reshape(N, T)
    tidx = np.asarray(inputs["time_idx"]).astype(np.int64)
    didx = np.asarray(inputs["day_idx"]).astype(np.int64)
    ei = np.asarray(inputs["edge_index"]).astype(np.int64)

    # ---- edges with self loops, table-row ids ----
    src = np.concatenate([ei[0], np.arange(N, dtype=np.int64)])
    dst = np.concatenate([ei[1], np.arange(N, dtype=np.int64)])
    trow = (src // NPC) * NPAD + (src % NPC)     # global gather-table row
    dcore = dst // NPC
    dloc = dst % NPC
    dblk = dloc // P
    dinb = dloc % P

    # order edges by (core, block, hi?) ; stable
    hi = (trow >= SPLIT).astype(np.int64)
    order = np.lexsort((hi, dblk, dcore))
    trow, dcore, dblk, dinb, hi = (a[order] for a in (trow, dcore, dblk, dinb, hi))

    # per (core, block) lo/hi counts
    nlo = np.zeros((CORES, NB), np.int64)
    nhi = np.zeros((CORES, NB), np.int64)
    key = (dcore * NB + dblk) * 2 + hi
    cnt = np.bincount(key, minlength=CORES * NB * 2).reshape(CORES, NB, 2)
    nlo, nhi = cnt[:, :, 0], cnt[:, :, 1]
    klo = np.maximum(1, -(-nlo.max(axis=0) // P))   # per-block chunk counts
    khi = np.maximum(1, -(-nhi.max(axis=0) // P))
    ktot = klo + khi
    KTOT = int(ktot.sum())
    off_lo = np.concatenate([[0], np.cumsum(klo)]).astype(int)
    off_hi = np.concatenate([[0], np.cumsum(khi)]).astype(int)
    off_kt = np.concatenate([[0], np.cumsum(ktot)]).astype(int)

    # per-core edge arrays
    idx_lo = np.zeros((CORES, 128, int(klo.sum()) * 8), np.int16)
    idx_hi = np.zeros((CORES, 128, int(khi.sum()) * 8), np.int16)
    dstloc = np.full((CORES, 128, KTOT), -1.0, f32)
    dstflat = np.full((CORES, 1, KTOT * P), -1.0, f32)

    # edge run boundaries per (core, block, hi)
    starts = np.concatenate([[0], np.cumsum(cnt.reshape(-1))]).astype(int)
    for c in range(CORES):
        for b in range(NB):
            k0 = (c * NB + b) * 2
            lo_s, lo_e = starts[k0], starts[k0 + 1]
            hi_s, hi_e = starts[k0 + 1], starts[k0 + 2]
            Llo, Lhi = int(klo[b]) * P, int(khi[b]) * P
            li = np.zeros(Llo, np.int64)
            li[: lo_e - lo_s] = trow[lo_s:lo_e]
            hii = np.zeros(Lhi, np.int64)
            hii[: hi_e - hi_s] = trow[hi_s:hi_e] - SPLIT
            idx_lo[c, :, off_lo[b] * 8 : off_lo[b + 1] * 8] = _wrap16(
                li.astype(np.int16))
            idx_hi[c, :, off_hi[b] * 8 : off_hi[b + 1] * 8] = _wrap16(
                hii.astype(np.int16))
            dl = np.full(Llo + Lhi, -1.0, f32)
            dl[: lo_e - lo_s] = dinb[lo_s:lo_e]
            dl[Llo : Llo + hi_e - hi_s] = dinb[hi_s:hi_e]
            dstloc[c, :, off_kt[b] : off_kt[b + 1]] = dl.reshape(-1, P).T
            dstflat[c, 0, off_kt[b] * P : off_kt[b + 1] * P] = dl

    # ---- per-core node arrays (padded to NPAD) ----
    def pad_nodes(a, fill=0.0):
        out = np.full((CORES, NPAD) + a.shape[1:], fill, a.dtype)
        for c in range(CORES):
            out[c, :NPC] = a[c * NPC : (c + 1) * NPC]
        return out

    xsT = pad_nodes(xs).transpose(0, 2, 1).copy()          # [C,16,NPAD]
    hsp = pad_nodes(hs)                                     # [C,NPAD,12]
    # hist broadcast-packed: [C, T, 128, HALF]
    hist_bc = np.empty((CORES, T, 128, HALF), f32)
    for c in range(CORES):
        h = hsp[c].T                                        # [T, NPAD]
        hist_bc[c, :, :64, :] = h[:, None, :HALF]
        hist_bc[c, :, 64:, :] = h[:, None, HALF:]
    tpad = pad_nodes(tidx)[:, None, :].astype(f32)          # [C,1,NPAD]
    dpad = (pad_nodes(didx).astype(f32) + 288.0)[:, None, :]

    # ---- weights (replicated) ----
    w_ih = np.asarray(inputs["w_ih"], f32)      # [192,1]
    w_hh = np.asarray(inputs["w_hh"], f32)      # [192,64]
    b_ih = np.asarray(inputs["b_ih"], f32)
    b_hh = np.asarray(inputs["b_hh"], f32)
    W1 = np.asarray(inputs["W1"], f32)          # [112,256]
    a1s = np.asarray(inputs["a1_src"], f32)     # [4,64]
    a1d = np.asarray(inputs["a1_dst"], f32)
    b1 = np.asarray(inputs["b1"], f32)          # [64]
    W2 = np.asarray(inputs["W2"], f32)          # [64,64]
    a2s = np.asarray(inputs["a2_src"], f32)     # [1,64]
    a2d = np.asarray(inputs["a2_dst"], f32)
    b2 = np.asarray(inputs["b2"], f32)

    whh_pack = np.zeros((128, 192), f32)
    for g in range(3):
        blkT = w_hh[g * 64 : (g + 1) * 64, :].T             # [64k,64m]
        whh_pack[:64, g * 64 : (g + 1) * 64] = blkT
        whh_pack[64:, g * 64 : (g + 1) * 64] = blkT
    dup = lambda v: np.tile(v.reshape(64, 1), (2, 1)).astype(f32)  # [128,1]
    wih_r, wih_z, wih_n = (dup(w_ih[g * 64 : (g + 1) * 64, 0]) for g in range(3))
    b_r = dup(b_ih[0:64] + b_hh[0:64])
    b_z = dup(b_ih[64:128] + b_hh[64:128])
    b_hhn = dup(b_hh[128:192])
    b_ihn = dup(b_ih[128:192])

    perm = np.concatenate([np.arange(48, 112), np.arange(16, 48),
                           np.arange(0, 16)])
    W1p = W1[perm].copy()                                   # [112,256]
    Wes = np.zeros((112, 8), f32)
    for h in range(4):
        Wes[:, h] = W1p[:, h * 64 : (h + 1) * 64] @ a1s[h]
        Wes[:, 4 + h] = W1p[:, h * 64 : (h + 1) * 64] @ a1d[h]
    Wes2 = np.stack([W2 @ a2s[0], W2 @ a2d[0]], axis=1)     # [64,2]

    time_emb = np.asarray(inputs["time_emb"], f32)
    day_emb = np.asarray(inputs["day_emb"], f32)
    emb_pack = np.zeros((295, 32), f32)
    emb_pack[:288, :16] = time_emb
    emb_pack[288:, 16:] = day_emb
    emb_pad = np.zeros((39, 32), f32)
    emb_pad[: 295 - 256] = emb_pack[256:295]

    iota_row = np.tile(np.arange(128, dtype=f32), (128, 1))
    iota_col3 = np.stack([np.arange(128, dtype=f32) + b for b in (0, 128, 256)],
                         axis=1)                            # [128,3]
    b1x4 = np.tile(4.0 * b1, (128, 1)).astype(f32)          # [128,64]
    b2bc = np.tile(b2, (128, 1)).astype(f32)

    shared = dict(
        whh_pack=whh_pack, wih_r=wih_r, wih_z=wih_z, wih_n=wih_n,
        b_r=b_r, b_z=b_z, b_hhn=b_hhn, b_ihn=b_ihn,
        W1p=W1p, Wes=Wes, W2w=W2.copy(), Wes2=Wes2,
        emb0=emb_pack[0:128].copy(), emb1=emb_pack[128:256].copy(),
        emb2=emb_pad, iota=iota_row, iota3=iota_col3, b1x4=b1x4, b2bc=b2bc,
    )
    in_maps = []
    for c in range(CORES):
        m = dict(shared)
        m.update(
            xsT=xsT[c], hist_bc=hist_bc[c], tpad=tpad[c], dpad=dpad[c],
            idx_lo=idx_lo[c], idx_hi=idx_hi[c], dstloc=dstloc[c],
            dstflat=dstflat[c],
        )
        in_maps.append(m)

    meta = dict(klo=klo.astype(int), khi=khi.astype(int),
                off_lo=off_lo, off_hi=off_hi, off_kt=off_kt,
                NPAD=NPAD, HALF=HALF, PAIRS=PAIRS)
    return in_maps, meta


def build_nc(cfg, meta, num_devices):
    import concourse.bass as bass
    import concourse.bacc as bacc
    import concourse.tile as tile
    import concourse.mybir as mybir
    from concourse.masks import make_identity

    f32 = mybir.dt.float32
    alu = mybir.AluOpType
    act = mybir.ActivationFunctionType
    N, CORES = cfg["N"], cfg["CORES"]
    NB, CT, SPLIT = cfg["NB"], cfg["CT"], cfg["SPLIT"]
    NPAD, HALF, PAIRS = meta["NPAD"], meta["HALF"], meta["PAIRS"]
    klo, khi = meta["klo"], meta["khi"]
    off_lo, off_hi, off_kt = meta["off_lo"], meta["off_hi"], meta["off_kt"]
    GPAD = NPAD * CORES
    NHI = GPAD - SPLIT
    assert 0 < NHI <= 32768

    nc = bacc.Bacc("TRN2", target_bir_lowering=False, debug=False,
                   num_devices=num_devices)
    dt = lambda n, s, d=f32: nc.dram_tensor(n, s, d, kind="ExternalInput").ap()

    t_xsT = dt("xsT", [16, NPAD])
    t_hist = dt("hist_bc", [T, 128, HALF])
    t_tpad = dt("tpad", [1, NPAD])
    t_dpad = dt("dpad", [1, NPAD])
    t_ilo = dt("idx_lo", [128, int(klo.sum()) * 8], mybir.dt.int16)
    t_ihi = dt("idx_hi", [128, int(khi.sum()) * 8], mybir.dt.int16)
    t_dl = dt("dstloc", [128, int(off_kt[-1])])
    t_df = dt("dstflat", [1, int(off_kt[-1]) * P])
    t_whh = dt("whh_pack", [128, 192])
    t_wsc = {k: dt(k, [128, 1]) for k in
             ("wih_r", "wih_z", "wih_n", "b_r", "b_z", "b_hhn", "b_ihn")}
    t_W1p = dt("W1p", [112, 256])
    t_Wes = dt("Wes", [112, 8])
    t_W2 = dt("W2w", [64, 64])
    t_Wes2 = dt("Wes2", [64, 2])
    t_emb = [dt("emb0", [128, 32]), dt("emb1", [128, 32]), dt("emb2", [39, 32])]
    t_iota = dt("iota", [128, 128])
    t_iota3 = dt("iota3", [128, 3])
    t_b1x4 = dt("b1x4", [128, 64])
    t_b2bc = dt("b2bc", [128, 64])
    t_y = nc.dram_tensor("y", [NPAD, 64], f32, kind="ExternalOutput").ap()

    with tile.TileContext(nc) as tc:
        with tc.tile_pool(name="const", bufs=1) as cpool, \
             tc.tile_pool(name="state", bufs=1) as spool, \
             tc.tile_pool(name="work", bufs=3) as wpool, \
             tc.tile_pool(name="gath", bufs=2) as gpool, \
             tc.tile_pool(name="psum", bufs=1, space="PSUM") as pp, \
             tc.tile_pool(name="dram", bufs=1, space="DRAM") as dpool:

            # ---------- constants into SBUF ----------
            def load_const(t_ap, shape, dtype=f32, tag=None):
                nm = tag or t_ap.tensor.name
                tl = cpool.tile(shape, dtype, tag=nm, name=nm)
                nc.sync.dma_start(tl[:], t_ap[:])
                return tl
            whh = load_const(t_whh, [128, 192])
            wsc = {k: load_const(v, [128, 1]) for k, v in t_wsc.items()}
            W1s = load_const(t_W1p, [112, 256])
            Wes_s = load_const(t_Wes, [112, 8])
            W2s = load_const(t_W2, [64, 64])
            Wes2_s = load_const(t_Wes2, [64, 2])
            embs = [load_const(t_emb[0], [128, 32]),
                    load_const(t_emb[1], [128, 32]),
                    load_const(t_emb[2], [39, 32])]
            iota = load_const(t_iota, [128, 128])
            iota3 = load_const(t_iota3, [128, 3])
            b1x4 = load_const(t_b1x4, [128, 64])
            b2bc = load_const(t_b2bc, [128, 64])
            ident = cpool.tile([128, 128], f32, tag="ident")
            make_identity(nc, ident)

            # ---------- persistent state ----------
            baseT = spool.tile([112, NPAD], f32, tag="bigbuf", name="baseT")
            x2sb = spool.tile([128, NB, 64], f32, tag="x2sb")
            ed1 = spool.tile([128, NB, 4], f32, tag="ed1")
            ed2 = spool.tile([128, NB, 1], f32, tag="ed2")
            hps = [spool.tile([128, CT], f32, tag=f"hp{p}", name=f"hp{p}")
                   for p in range(PAIRS)]
            for hp in hps:
                nc.vector.memset(hp[:], 0.0)
            nc.sync.dma_start(baseT[96:112, :], t_xsT[:])

            # ---------- GRU ----------
            for t in range(T):
                for p in range(PAIRS):
                    hp = hps[p]
                    xt = wpool.tile([128, CT], f32, tag="xt")
                    nc.sync.dma_start(xt[:], t_hist[t, :, p * CT : (p + 1) * CT])
                    xp = xt[:, :]
                    ps_r = pp.tile([128, CT], f32, space="PSUM", tag="ps_r")
                    ps_z = pp.tile([128, CT], f32, space="PSUM", tag="ps_z")
                    ps_n = pp.tile([128, CT], f32, space="PSUM", tag="ps_n")
                    for g, ps in enumerate((ps_r, ps_z, ps_n)):
                        for base in (0, 64):
                            nc.tensor.matmul(
                                ps[base : base + 64, :],
                                lhsT=whh[base : base + 64, g * 64 : (g + 1) * 64],
                                rhs=hp[base : base + 64, :],
                                start=True, stop=True,
                                tile_position=(base, base))
                    rt = wpool.tile([128, CT], f32, tag="rt", bufs=2)
                    zt = wpool.tile([128, CT], f32, tag="zt", bufs=2)
                    nt = wpool.tile([128, CT], f32, tag="nt", bufs=2)
                    gn = wpool.tile([128, CT], f32, tag="gn", bufs=2)
                    # r,z pre-acts: (x*wih)+psum then sigmoid(+bias)
                    nc.vector.scalar_tensor_tensor(
                        out=rt[:], in0=xp, scalar=wsc["wih_r"][:, 0:1],
                        in1=ps_r[:], op0=alu.mult, op1=alu.add)
                    nc.scalar.activation(rt[:], rt[:], act.Sigmoid,
                                         bias=wsc["b_r"][:, 0:1])
                    nc.vector.scalar_tensor_tensor(
                        out=zt[:], in0=xp, scalar=wsc["wih_z"][:, 0:1],
                        in1=ps_z[:], op0=alu.mult, op1=alu.add)
                    nc.scalar.activation(zt[:], zt[:], act.Sigmoid,
                                         bias=wsc["b_z"][:, 0:1])
                    # gn = x*wih_n + b_ihn  (gpsimd)
                    nc.gpsimd.tensor_scalar(
                        out=gn[:], in0=xp, scalar1=wsc["wih_n"][:, 0:1],
                        scalar2=wsc["b_ihn"][:, 0:1],
                        op0=alu.mult, op1=alu.add)
                    # tmp = (hn+b_hhn)*r ; nin = tmp+gn ; ncand = tanh
                    nc.vector.scalar_tensor_tensor(
                        out=nt[:], in0=ps_n[:], scalar=wsc["b_hhn"][:, 0:1],
                        in1=rt[:], op0=alu.add, op1=alu.mult)
                    nc.vector.tensor_tensor(out=nt[:], in0=nt[:], in1=gn[:],
                                            op=alu.add)
                    nc.scalar.activation(nt[:], nt[:], act.Tanh)
                    # h = ncand + z*(h-ncand)
                    dt_ = wpool.tile([128, CT], f32, tag="dt", bufs=2)
                    nc.gpsimd.tensor_tensor(out=dt_[:], in0=hp[:], in1=nt[:],
                                            op=alu.subtract)
                    nc.gpsimd.tensor_tensor(out=dt_[:], in0=zt[:], in1=dt_[:],
                                            op=alu.mult)
                    nc.vector.tensor_tensor(out=hp[:], in0=nt[:], in1=dt_[:],
                                            op=alu.add)

            # GRU -> baseT rows 0:64 (A direct, B via partition-shift DMA)
            for p in range(PAIRS):
                nc.vector.tensor_copy(
                    baseT[0:64, p * CT : (p + 1) * CT], hps[p][0:64, :])
                nc.sync.dma_start(
                    baseT[0:64, HALF + p * CT : HALF + (p + 1) * CT],
                    hps[p][64:128, :])

            # ---------- time/day one-hot -> baseT rows 64:96 ----------
            NCT = NPAD // CT
            for i in range(NCT):
                sl = slice(i * CT, (i + 1) * CT)
                tb1 = wpool.tile([1, CT], f32, tag="tb1")
                db1 = wpool.tile([1, CT], f32, tag="db1")
                nc.sync.dma_start(tb1[:], t_tpad[:, sl])
                nc.sync.dma_start(db1[:], t_dpad[:, sl])
                tb = wpool.tile([128, CT], f32, tag="tbq")
                db = wpool.tile([128, CT], f32, tag="dbq")
                nc.gpsimd.partition_broadcast(tb[:], tb1[:])
                nc.gpsimd.partition_broadcast(db[:], db1[:])
                ps_tf = pp.tile([32, CT], f32, space="PSUM", tag="ps_med")
                for k in range(3):
                    rows = 128 if k < 2 else 39
                    oh = wpool.tile([rows, CT], f32, tag="oh")
                    if k < 2:
                        nc.vector.tensor_scalar(
                            out=oh[:], in0=tb[0:rows, :],
                            scalar1=iota3[0:rows, k : k + 1], scalar2=None,
                            op0=alu.is_equal)
                    else:
                        # rows 0:32 are time ids 256..287; rows 32:39 day ids
                        nc.vector.tensor_scalar(
                            out=oh[0:32, :], in0=tb[0:32, :],
                            scalar1=iota3[0:32, k : k + 1], scalar2=None,
                            op0=alu.is_equal)
                        nc.vector.tensor_scalar(
                            out=oh[32:39, :], in0=db[32:39, :],
                            scalar1=iota3[32:39, k : k + 1], scalar2=None,
                            op0=alu.is_equal)
                    nc.tensor.matmul(ps_tf[:], lhsT=embs[k][:, :], rhs=oh[:],
                                     start=(k == 0), stop=(k == 2))
                nc.vector.tensor_copy(baseT[64:96, sl], ps_tf[:])

            # ---------- layer tables + edge phase ----------
            cc1_in = dpool.tile([NPAD, D1], f32, tag="cc1_in")
            cc1_out = dpool.tile([GPAD, D1], f32, tag="cc1_out",
                                 addr_space="Shared")
            cc2_in = dpool.tile([NPAD, D2], f32, tag="cc2_in")
            cc2_out = dpool.tile([GPAD, D2], f32, tag="cc2_out",
                                 addr_space="Shared")

            # layer-1 node tables
            for g in range(NB):
                sl = slice(g * 128, (g + 1) * 128)
                ps_h1 = pp.tile([128, 256], f32, space="PSUM", tag="ps_big")
                ps_es = pp.tile([128, 8], f32, space="PSUM", tag="ps_small")
                nc.tensor.matmul(ps_h1[:], lhsT=baseT[:, sl], rhs=W1s[:],
                                 start=True, stop=True)
                nc.tensor.matmul(ps_es[:], lhsT=baseT[:, sl], rhs=Wes_s[:],
                                 start=True, stop=True)
                tb_t = wpool.tile([128, D1], f32, tag="tb1l")
                # interleaved [h_h(64),1.0]x4, es(4)
                nc.vector.tensor_copy(
                    tb_t[:, 0:260].rearrange("p (h c) -> p h c", c=65)[:, :, 0:64],
                    ps_h1[:].rearrange("p (h c) -> p h c", c=64))
                nc.vector.memset(
                    tb_t[:, 0:260].rearrange("p (h c) -> p h c", c=65)[:, :, 64:65],
                    1.0)
                nc.vector.tensor_copy(tb_t[:, 260:264], ps_es[:, 0:4])
                nc.vector.tensor_copy(ed1[:, g, :], ps_es[:, 4:8])
                nc.vector.memset(tb_t[:, 264:D1], 0.0)
                nc.sync.dma_start(cc1_in[sl, :], tb_t[:])

            nc.gpsimd.collective_compute(
                "AllGather", alu.bypass,
                replica_groups=[list(range(num_devices))],
                ins=[cc1_in[:].opt()], outs=[cc1_out[:].opt()])

            # ---------- edge phase (shared for both layers) ----------
            def edge_layer(cc_out, D, NH, ed_sb, out_cb):
                """NH = heads; out_cb(b, psum_blk) consumes the block result."""
                W65 = NH * 64 + NH  # msg width
                for b in range(NB):
                    kl, kh = int(klo[b]), int(khi[b])
                    kt = kl + kh
                    L = kt * P
                    g = gpool.tile([128, kt, D], f32, tag="gt")
                    ilo_t = wpool.tile([128, kl * 8], mybir.dt.int16, tag="ilo")
                    ihi_t = wpool.tile([128, kh * 8], mybir.dt.int16, tag="ihi")
                    dl_t = wpool.tile([128, kt], f32, tag="dlt")
                    df_t = wpool.tile([1, L], f32, tag="dft", bufs=1)
                    nc.sync.dma_start(
                        ilo_t[:], t_ilo[:, off_lo[b] * 8 : off_lo[b + 1] * 8])
                    nc.sync.dma_start(
                        ihi_t[:], t_ihi[:, off_hi[b] * 8 : off_hi[b + 1] * 8])
                    nc.sync.dma_start(
                        dl_t[:], t_dl[:, off_kt[b] : off_kt[b + 1]])
                    nc.sync.dma_start(
                        df_t[:], t_df[:, off_kt[b] * P : off_kt[b + 1] * P])
                    GMAX = 8  # max chunks (1024 idxs) per dma_gather
                    for s0 in range(0, kl, GMAX):
                        kk = min(GMAX, kl - s0)
                        nc.gpsimd.dma_gather(
                            g[:, s0 : s0 + kk, :], cc_out[:],
                            ilo_t[:, s0 * 8 : (s0 + kk) * 8],
                            kk * P, kk * P, D)
                    for s0 in range(0, kh, GMAX):
                        kk = min(GMAX, kh - s0)
                        nc.gpsimd.dma_gather(
                            g[:, kl + s0 : kl + s0 + kk, :], cc_out[SPLIT:, :],
                            ihi_t[:, s0 * 8 : (s0 + kk) * 8],
                            kk * P, kk * P, D)
                    # dst broadcast + ed gather via transposed one-hot matmul
                    dbc = gpool.tile([128, L], f32, tag="dbc", bufs=1)
                    nc.gpsimd.partition_broadcast(dbc[:], df_t[:])
                    ps_ed = pp.tile([128, kt * NH], f32, space="PSUM",
                                    tag="ps_med")
                    for c in range(kt):
                        w2c = wpool.tile([128, 128], f32, tag="w2c")
                        nc.vector.tensor_scalar(
                            out=w2c[:], in0=dbc[:, c * P : (c + 1) * P],
                            scalar1=iota3[:, 0:1], scalar2=None,
                            op0=alu.is_equal)
                        nc.tensor.matmul(
                            ps_ed[:, c * NH : (c + 1) * NH], lhsT=w2c[:],
                            rhs=ed_sb[:, b, :], start=True, stop=True)
                    # logits -> ex
                    ex = wpool.tile([128, kt, NH], f32, tag="ex")
                    es_ap = g[:, :, NH * 65 : NH * 65 + NH]
                    nc.vector.tensor_tensor(
                        out=ex[:], in0=es_ap,
                        in1=ps_ed[:].rearrange("p (c h) -> p c h", h=NH),
                        op=alu.add)
                    nc.vector.scalar_tensor_tensor(
                        out=ex[:], in0=ex[:], scalar=0.2, in1=ex[:],
                        op0=alu.mult, op1=alu.max)
                    nc.scalar.activation(ex[:], ex[:], act.Exp)
                    # messages + segment-sum
                    ps_blk = pp.tile([128, W65], f32, space="PSUM",
                                      tag="ps_blk")
                    for c in range(kt):
                        msg = wpool.tile([128, W65], f32, tag="msg")
                        nc.vector.tensor_tensor(
                            out=msg[:], in0=g[:, c, 0:W65],
                            in1=ex[:, c, :].to_broadcast([128, NH, 65]),
                            op=alu.mult)
                        wc = wpool.tile([128, 128], f32, tag="wc")
                        nc.vector.tensor_scalar(
                            out=wc[:], in0=iota[:],
                            scalar1=dl_t[:, c : c + 1], scalar2=None,
                            op0=alu.is_equal)
                        nc.tensor.matmul(ps_blk[:], lhsT=wc[:], rhs=msg[:],
                                         start=(c == 0), stop=(c == kt - 1))
                    out_cb(b, ps_blk)

            # layer-1 block output -> relu'd x2
            def l1_out(b, ps_blk):
                rden = wpool.tile([128, 4], f32, tag="rden")
                nc.vector.tensor_scalar(
                    out=rden[:], in0=ps_blk[:, 0:260].rearrange(
                        "p (h c) -> p h c", c=65)[:, :, 64],
                    scalar1=1e-30, scalar2=None, op0=alu.add)
                nc.vector.reciprocal(rden[:], rden[:])
                o = wpool.tile([128, 64], f32, tag="o1")
                nc.vector.scalar_tensor_tensor(
                    out=o[:], in0=ps_blk[:, 0:64], scalar=rden[:, 0:1],
                    in1=b1x4[:], op0=alu.mult, op1=alu.add)
                for h in range(1, 4):
                    nc.vector.scalar_tensor_tensor(
                        out=o[:], in0=ps_blk[:, h * 65 : h * 65 + 64],
                        scalar=rden[:, h : h + 1], in1=o[:],
                        op0=alu.mult, op1=alu.add)
                nc.scalar.activation(x2sb[:, b, :], o[:], act.Relu, scale=0.25)

            edge_layer(cc1_out, D1, 4, ed1, l1_out)

            # ---------- layer-2 node tables ----------
            x2T = spool.tile([64, NPAD], f32, tag="bigbuf", name="x2T")
            for g in range(NB):
                sl = slice(g * 128, (g + 1) * 128)
                ps_t = pp.tile([128, 128], f32, space="PSUM", tag="ps_big")
                nc.tensor.transpose(ps_t[0:64, :], x2sb[:, g, :], ident[:])
                nc.vector.tensor_copy(x2T[:, sl], ps_t[0:64, :])
            for g in range(NB):
                sl = slice(g * 128, (g + 1) * 128)
                ps_h2 = pp.tile([128, 64], f32, space="PSUM", tag="ps_big")
                ps_e2 = pp.tile([128, 2], f32, space="PSUM", tag="ps_small")
                nc.tensor.matmul(ps_h2[:], lhsT=x2T[:, sl], rhs=W2s[:],
                                 start=True, stop=True)
                nc.tensor.matmul(ps_e2[:], lhsT=x2T[:, sl], rhs=Wes2_s[:],
                                 start=True, stop=True)
                tb2 = wpool.tile([128, D2], f32, tag="tb2l")
                nc.vector.tensor_copy(tb2[:, 0:64], ps_h2[:])
                nc.vector.memset(tb2[:, 64:65], 1.0)
                nc.vector.tensor_copy(tb2[:, 65:66], ps_e2[:, 0:1])
                nc.vector.tensor_copy(ed2[:, g, :], ps_e2[:, 1:2])
                nc.vector.memset(tb2[:, 66:D2], 0.0)
                nc.sync.dma_start(cc2_in[sl, :], tb2[:])

            nc.gpsimd.collective_compute(
                "AllGather", alu.bypass,
                replica_groups=[list(range(num_devices))],
                ins=[cc2_in[:].opt()], outs=[cc2_out[:].opt()])

            def l2_out(b, ps_blk):
                rden = wpool.tile([128, 1], f32, tag="rden2")
                nc.vector.tensor_scalar(
                    out=rden[:], in0=ps_blk[:, 64:65],
                    scalar1=1e-30, scalar2=None, op0=alu.add)
                nc.vector.reciprocal(rden[:], rden[:])
                o = wpool.tile([128, 64], f32, tag="o2")
                nc.vector.scalar_tensor_tensor(
                    out=o[:], in0=ps_blk[:, 0:64], scalar=rden[:, 0:1],
                    in1=b2bc[:], op0=alu.mult, op1=alu.add)
                yo = wpool.tile([128, 64], f32, tag="yo")
                nc.scalar.activation(yo[:], o[:], act.Relu)
                nc.sync.dma_start(t_y[b * 128 : (b + 1) * 128, :], yo[:])

            edge_layer(cc2_out, D2, 1, ed2, l2_out)

    nc.compile()
    return nc


def _run(inputs, cfg, sim=False):
    sys.path.insert(0, "/opt/trn_rl_repo")
    import numpy as _np
    in_maps, meta = host_prep(inputs, cfg)
    nc = build_nc(cfg, meta, cfg["CORES"])
    if sim:
        from concourse.bass_interp import MultiCoreSim
        ms = MultiCoreSim(nc, num_cores=cfg["CORES"], trace=False,
                          require_finite=False, require_nnan=False)
        for c, core in enumerate(ms.cores.values()):
            for k, v in in_maps[c].items():
                core.tensor(k)[:] = v
        ms.simulate(check_with_hw=False)
        outs = [core.tensor("y").copy() for core in ms.cores.values()]
    else:
        from concourse import bass_utils
        res = bass_utils.run_bass_kernel_spmd(
            nc, in_maps, core_ids=list(range(cfg["CORES"])))
        outs = [res.results[c]["y"] for c in range(cfg["CORES"])]
    NPC = cfg["NODES_PC"]
    return _np.concatenate([o[:NPC] for o in outs], axis=0)


def kernel(**inputs):
    return _run(inputs, FULL_CFG, sim=False)
